# revision 8
# baseline (speedup 1.0000x reference)
"""Trainium2 Bass kernel for AttentionBlock (B=8, C=256, L=2048), data-parallel
over batch across 8 NeuronCores.

Math (one batch per core, x: [C, L]):
    scores^T = x^T M x + (u.x) 1^T   with  M = Wq^T Wk,  u = Wk^T bq / sqrt(C)
        (host precomputes the tiny [C,C] M and [C] u; bk drops out of softmax
         exactly - it only adds a per-query constant)
    pT = exp(scores^T / sqrt(C) + ux)        [m, l], m on partitions
    denom = ones^T acc(pT)                    (running bf16 accumulator on DVE)
    ctx = vT^T pT,  vT = x^T Wv^T
    out = ctx * (1/denom) + (x + bv)

All matmuls are bf16 with fp32 PSUM accumulation; softmax skips
max-subtraction (|scores| < ~6, exact in fp32).

The transposed-scores orientation means exp's PSUM->SBUF eviction directly
produces the layout the context matmul needs - no on-chip transposes of the
LxL matrix. The q/k projections are fused into the single w = M x projection.

Schedule notes:
 - warmup matmuls on a constant tile release the PE HAM clock-gate early
 - x arrives as bf16 (compute) first, sliced across both HWDGE queues with
   the weights interleaved ahead of later slices; fp32 x (residual) is
   issued late - it is only needed by the epilogue
 - the per-query bias row u.x is built on PE (M=1 matmuls), bounced through
   a DRAM scratch to transpose into [m-chunk, 1] layout for exp's bias slot
 - the denominator accumulates pT chunks on DVE during the scores phase
 - context quarter 0 is emitted before the denominator matmuls so the PE
   rolls straight from scores into context; epilogue slices alternate DMA
   queues, the last quarter extra-fine so the final store drains fast
"""

import os
import numpy as np
import ml_dtypes

import concourse.bass as bass
import concourse.tile as tile
from concourse import bacc, mybir
from concourse.bass_utils import run_bass_kernel_spmd

B, C, L = 8, 256, 2048
P = 128                 # partitions
NCC = C // P            # 2 channel chunks
NMC = L // P            # 16 m-chunks (key blocks)
NB = 512                # matmul moving free dim
NLN = L // NB           # 4 col slices of 512
HALF = 1024
SCALE = float(C) ** -0.5
WARMUP_MMS = 8

F32 = mybir.dt.float32
BF16 = mybir.dt.bfloat16

_COMPILED = None


def build_nc():
    nc = bacc.Bacc("TRN2", target_bir_lowering=False, debug=False, num_devices=8)

    x_d = nc.dram_tensor("x", [C, L], F32, kind="ExternalInput").ap()
    xbf_d = nc.dram_tensor("xbf", [C, L], BF16, kind="ExternalInput").ap()
    mt_d = nc.dram_tensor("mt", [C, C], BF16, kind="ExternalInput").ap()
    wvt_d = nc.dram_tensor("wvt", [C, C], BF16, kind="ExternalInput").ap()
    u_d = nc.dram_tensor("u", [C, 1], BF16, kind="ExternalInput").ap()
    bv_d = nc.dram_tensor("bv", [C, 1], F32, kind="ExternalInput").ap()
    out_d = nc.dram_tensor("out", [C, L], F32, kind="ExternalOutput").ap()
    uxs_d = nc.dram_tensor("uxs", [1, L], F32).ap()      # scratch bounce

    with tile.TileContext(nc) as tc:
        with (
            tc.tile_pool(name="const", bufs=1) as const,
            tc.tile_pool(name="data", bufs=1) as data,
            tc.tile_pool(name="evict", bufs=4) as evict,
        ):
            # ---- constants ----
            ones_bf = const.tile([P, NB], BF16)
            nc.vector.memset(ones_bf[:], 1.0)

            x_bf = [data.tile([P, L], BF16, tag=f"xbf{c}", name=f"xbf{c}") for c in range(NCC)]
            mt = const.tile([P, NCC, C], BF16, tag="mt")
            wvt = const.tile([P, NCC, C], BF16, tag="wvt")
            u_sb = const.tile([P, NCC, 1], BF16, tag="u")
            bv_sb = const.tile([P, NCC, 1], F32, tag="bv")

            # first l-slice of bf16 x on both queues, then the weights,
            # then the remaining slices
            def xbf_dma(ln, cc):
                cols = slice(ln * NB, (ln + 1) * NB)
                rows = slice(cc * P, (cc + 1) * P)
                eng = nc.sync if cc == 0 else nc.scalar
                eng.dma_start(out=x_bf[cc][:, cols], in_=xbf_d[rows, cols])

            for piece in range(2):
                cols = slice(piece * 256, (piece + 1) * 256)
                nc.sync.dma_start(out=x_bf[0][:, cols], in_=xbf_d[0:P, cols])
                nc.scalar.dma_start(out=x_bf[1][:, cols], in_=xbf_d[P:C, cols])
            nc.sync.dma_start(out=mt[:], in_=mt_d.rearrange("(a p) c -> p a c", p=P))
            nc.scalar.dma_start(out=wvt[:], in_=wvt_d.rearrange("(a p) c -> p a c", p=P))
            nc.scalar.dma_start(out=u_sb[:], in_=u_d.rearrange("(a p) o -> p a o", p=P))
            nc.scalar.dma_start(out=bv_sb[:], in_=bv_d.rearrange("(a p) o -> p a o", p=P))
            for ln in range(1, NLN):
                xbf_dma(ln, 0)
                cols = slice(ln * NB, (ln + 1) * NB)
                nc.gpsimd.dma_start(out=x_bf[1][:, cols], in_=xbf_d[P:C, cols])

            x_f = [data.tile([P, L], F32, tag=f"xf{c}", name=f"xf{c}") for c in range(NCC)]
            w_bf = [data.tile([P, L], BF16, tag=f"w{c}", name=f"w{c}") for c in range(NCC)]
            vT_bf = data.tile([P, NMC, C], BF16, tag="vT")
            pT_bf = data.tile([P, NMC, L], BF16, tag="pT")
            dacc = data.tile([P, L], BF16, tag="dacc")
            recip = data.tile([P, L], F32, tag="recip")
            ux_row = data.tile([1, L], F32, tag="uxrow")
            ux_col = data.tile([P, NMC, 1], F32, tag="uxcol")

            # ---- phase 1: projections (psA released before scores) ----
            with tc.tile_pool(name="psA", bufs=3, space=bass.MemorySpace.PSUM) as psA:
                # PE warmup on the constant tile while x streams in
                warm = psA.tile([P, HALF], F32, tag="p", name="warm")
                for _ in range(WARMUP_MMS):
                    nc.tensor.matmul(warm[:, 0:NB], ones_bf[:, 0:P],
                                     ones_bf[:], start=True, stop=True)

                # ux row = u.x (per-query score bias) per 512-slice, bounced
                # through DRAM on the idle GpSimd queue to transpose into
                # [m-chunk, 1] layout well before the first exp needs it
                def ux_chain(ln):
                    cols = slice(ln * NB, (ln + 1) * NB)
                    up = psA.tile([1, NB], F32, tag="ux", name="up", bufs=2)
                    for cc in range(NCC):
                        nc.tensor.matmul(up[0:1, :], u_sb[:, cc, :],
                                         x_bf[cc][:, cols],
                                         start=(cc == 0), stop=(cc == NCC - 1))
                    nc.vector.tensor_copy(out=ux_row[0:1, cols], in_=up[:])
                    nc.gpsimd.dma_start(out=uxs_d[0:1, cols], in_=ux_row[0:1, cols])
                    nc.gpsimd.dma_start(
                        out=ux_col[:, ln * 4:(ln + 1) * 4, :],
                        in_=uxs_d[0:1, cols].rearrange("o (a p) -> p a o", p=P))

                ux_chain(0)
                ux_chain(1)

                # w = M x in 1024-col halves (scores only need w's m-slices)
                for h in range(2):
                    hcols = slice(h * HALF, (h + 1) * HALF)
                    for oc in range(NCC):
                        wp = psA.tile([P, HALF], F32, tag="p", name="wp")
                        for cc in range(NCC):
                            lhsT = mt[:, cc, oc * P:(oc + 1) * P]
                            for ln in range(2):
                                c0 = h * HALF + ln * NB
                                nc.tensor.matmul(
                                    wp[:, ln * NB:(ln + 1) * NB],
                                    lhsT, x_bf[cc][:, c0:c0 + NB],
                                    start=(cc == 0), stop=(cc == NCC - 1))
                        nc.scalar.copy(out=w_bf[oc][:, hcols], in_=wp[:])
                # vT[m, c] = sum_c' x[c', m] WvT[c', c]; 4 m-chunks per tile
                for qh in range(4):
                    vp = psA.tile([P, HALF], F32, tag="p", name="vp")
                    for i4 in range(4):
                        mc = qh * 4 + i4
                        for cc in range(NCC):
                            nc.tensor.matmul(
                                vp[:, i4 * C:(i4 + 1) * C],
                                x_bf[cc][:, mc * P:(mc + 1) * P], wvt[:, cc, :],
                                start=(cc == 0), stop=(cc == NCC - 1))
                    nc.vector.tensor_copy(out=vT_bf[:, qh * 4:(qh + 1) * 4, :],
                                          in_=vp[:])
                ux_chain(2)
                ux_chain(3)

            # ---- phase 2: transposed scores, exp, running denominator ----
            with tc.tile_pool(name="psS", bufs=2, space=bass.MemorySpace.PSUM) as psS:
                for mc in range(NMC):
                    mrows = slice(mc * P, (mc + 1) * P)
                    s = psS.tile([P, L], F32, tag="s", name="s")
                    for cc in range(NCC):
                        lhsT = w_bf[cc][:, mrows]
                        for ln in range(NLN):
                            col = ln * NB
                            nc.tensor.matmul(
                                s[:, col:col + NB],
                                lhsT, x_bf[cc][:, col:col + NB],
                                start=(cc == 0), stop=(cc == NCC - 1))
                    nc.scalar.activation(
                        out=pT_bf[:, mc, :],
                        in_=s[:], func=mybir.ActivationFunctionType.Exp,
                        scale=SCALE, bias=ux_col[:, mc, :])
                    if mc == 0:
                        nc.vector.tensor_copy(out=dacc[:], in_=pT_bf[:, 0, :])
                    else:
                        nc.vector.tensor_add(dacc[:], dacc[:], pT_bf[:, mc, :])

            # fp32 x for the residual - not needed until the epilogue
            for cc in range(NCC):
                rows = slice(cc * P, (cc + 1) * P)
                nc.sync.dma_start(out=x_f[cc][:], in_=x_d[rows, :])
                nc.vector.tensor_scalar_add(out=x_f[cc][:], in0=x_f[cc][:],
                                            scalar1=bv_sb[:, cc, :])

            # ---- phase 3: context quarters + denominator + epilogue ----
            with tc.tile_pool(name="psC", bufs=2, space=bass.MemorySpace.PSUM) as psC:
                ctx_t = {}

                def ctx_mms(qt):
                    cols = slice(qt * NB, (qt + 1) * NB)
                    for cc in range(NCC):
                        ct = psC.tile([P, NB], F32, tag=f"ctx{cc}", name=f"ctx{cc}")
                        ctx_t[(qt, cc)] = ct
                        for mc in range(NMC):
                            nc.tensor.matmul(
                                ct[:],
                                vT_bf[:, mc, cc * P:(cc + 1) * P],
                                pT_bf[:, mc, cols],
                                start=(mc == 0), stop=(mc == NMC - 1))

                def ctx_evict(qt, nsub):
                    sub = NB // nsub
                    for cc in range(NCC):
                        rows = slice(cc * P, (cc + 1) * P)
                        for si in range(nsub):
                            c0 = qt * NB + si * sub
                            cols = slice(c0, c0 + sub)
                            pcols = slice(si * sub, (si + 1) * sub)
                            t = evict.tile([P, sub], F32, tag="t", name="t")
                            nc.vector.tensor_mul(t[:], ctx_t[(qt, cc)][:, pcols],
                                                 recip[:, cols])
                            o = evict.tile([P, sub], F32, tag="o", name="o")
                            nc.vector.tensor_add(o[:], t[:], x_f[cc][:, cols])
                            eng = nc.sync if (cc + si) % 2 == 0 else nc.scalar
                            eng.dma_start(out=out_d[rows, cols], in_=o[:])

                # quarter 0 accumulates while the denominator finishes on DVE
                ctx_mms(0)
                for ln in range(NLN):
                    cols = slice(ln * NB, (ln + 1) * NB)
                    ds = psC.tile([P, NB], F32, tag=f"d{ln}", name=f"d{ln}",
                                  bufs=1)
                    nc.tensor.matmul(ds[:], ones_bf[:, 0:P], dacc[:, cols],
                                     start=True, stop=True)
                    nc.vector.reciprocal_approx_fast(out=recip[:, cols],
                                                     in_=ds[:])
                for qt in range(1, NLN):
                    ctx_mms(qt)
                    ctx_evict(qt - 1, 2)
                ctx_evict(NLN - 1, 4)

    nc.compile()
    return nc


def get_compiled():
    global _COMPILED
    if _COMPILED is None:
        _COMPILED = build_nc()
    return _COMPILED


def make_in_maps(inputs):
    x = np.ascontiguousarray(np.asarray(inputs["x"], dtype=np.float32))
    Wq = np.asarray(inputs["Wq"], np.float32)
    Wk = np.asarray(inputs["Wk"], np.float32)
    M = Wq.T @ Wk                                   # [c', c'']: scores = x^T M x
    u = SCALE * (Wk.T @ np.asarray(inputs["bq"], np.float32))
    shared = {
        "mt": np.ascontiguousarray(M.T).astype(ml_dtypes.bfloat16),
        "wvt": np.ascontiguousarray(
            np.asarray(inputs["Wv"], np.float32).T).astype(ml_dtypes.bfloat16),
        "u": u.reshape(C, 1).astype(ml_dtypes.bfloat16),
        "bv": np.asarray(inputs["bv"], np.float32).reshape(C, 1),
    }
    return [{"x": x[i], "xbf": x[i].astype(ml_dtypes.bfloat16), **shared}
            for i in range(B)]


def run(inputs, trace=False, **kwargs):
    nc = get_compiled()
    res = run_bass_kernel_spmd(nc, make_in_maps(inputs),
                               core_ids=list(range(B)), trace=trace, **kwargs)
    out = np.stack([res.results[i]["out"] for i in range(B)], axis=0)
    return out.astype(np.float32), res


def kernel(**inputs):
    out, _ = run(inputs)
    return out
